# revision 6
# baseline (speedup 1.0000x reference)
"""ChildSum TreeLSTM op on 8 Trainium2 NeuronCores (Bass/Tile).

Strategy (per spec sharding hint): partition nodes across the 8 cores,
replicate the small weights, shard edges by destination node so each
core's segment-sum is local. The h/c node-state table is replicated in
every core's HBM so child gathers (h[src], c[src]) are local indirect
DMAs — no cross-core halo exchange is needed.

Per core (25 000 nodes = 196 blocks of 128):
  - host pre-groups edges by destination block; each block gets a fixed
    tile capacity (max over cores, so the single SPMD program fits all
    cores), tiles of <=128 edges.
  - per edge tile: indirect-gather hc[src] rows (bf16), PE-transpose the
    h half, f = sigmoid(h_src @ U_f^T + b) on PE+ACT, fc = f * c_src on
    DVE, then two selector matmuls on PE turn edges into per-block
    h_sum^T (feat-major) and c_agg (node-major) in PSUM.
  - per block: iou_t = [x | h_sum] @ [W_iouf[:768] ; U_iou]^T via four
    stationary loads (xT and h_sum^T tiles) against the replicated
    weight matrix, accumulated in PSUM.
  - per 2-block group: gates on ACT (sigmoid/tanh), products on DVE,
    results DMA'd to the core's node-shard output.

All matmul inputs are bf16 (fp32 accumulate in PSUM); activations and
outputs are fp32.
"""

import numpy as np
import ml_dtypes

import concourse.bass as bass
import concourse.tile as tile
import concourse.mybir as mybir
from concourse import bacc
from concourse.bass_utils import run_bass_kernel_spmd
from concourse.masks import make_identity
from concourse.mybir import ActivationFunctionType as AF

F32 = mybir.dt.float32
BF16 = mybir.dt.bfloat16
I32 = mybir.dt.int32
BF = ml_dtypes.bfloat16

NC = 8          # cores
N = 200000      # nodes
D = 256         # feature dim
NPC = N // NC   # nodes per core
P = 128
NB = (NPC + P - 1) // P          # node blocks per core (196)
NPAD = NB * P                    # padded nodes per core (25088)
NGRP = NB // 2                   # apply groups (2 blocks each)
XGRP = NB // 4                   # xt DMA groups (4 blocks each)
DEBUG = False


def _build_edge_structure(src, dst):
    """Group edges by (core, dest block); build shared tile structure and
    per-core padded index columns."""
    src = np.asarray(src).astype(np.int64).ravel()
    dst = np.asarray(dst).astype(np.int64).ravel()
    core_of = dst // NPC
    loc = dst % NPC
    blk_of = loc // P
    dstl = loc % P
    cb = core_of * NB + blk_of
    cnt = np.bincount(cb, minlength=NC * NB).reshape(NC, NB)
    maxcnt = cnt.max(axis=0)
    cap = ((np.maximum(maxcnt, 1) + 31) // 32) * 32  # multiple of 32, >=32

    # tiles per block: [128]*k + [rem]
    tile_sizes = []
    tile_block = []
    blockcol0 = np.zeros(NB, np.int64)
    for b in range(NB):
        blockcol0[b] = len(tile_sizes)
        c = int(cap[b])
        while c > 0:
            t = min(c, P)
            tile_sizes.append(t)
            tile_block.append(b)
            c -= t
    T = len(tile_sizes)

    order = np.argsort(cb, kind="stable")
    cb_s = cb[order]
    starts = np.zeros(NC * NB + 1, np.int64)
    np.cumsum(cnt.ravel(), out=starts[1:])
    rank = np.arange(len(src)) - starts[cb_s]
    core_s = core_of[order]
    blk_s = blk_of[order]
    col_idx = blockcol0[blk_s] + rank // P
    row_idx = rank % P

    srccols = np.zeros((NC, P, T), np.int32)
    dstcols = np.full((NC, P, T), -1, np.int32)
    srccols[core_s, row_idx, col_idx] = src[order].astype(np.int32)
    dstcols[core_s, row_idx, col_idx] = dstl[order].astype(np.int32)
    return {
        "tile_sizes": tile_sizes,
        "tile_block": tile_block,
        "blockcol0": blockcol0,
        "T": T,
        "srccols": srccols,
        "dstcols": dstcols,
    }


def _build_bass(T, tile_sizes, tile_block, has_biou, has_ufb):
    nc = bacc.Bacc("TRN2", target_bir_lowering=False, debug=False,
                   num_devices=NC)

    hc_d = nc.dram_tensor("hc", [N, 2 * D], BF16, kind="ExternalInput")
    xt_d = nc.dram_tensor("xt", [XGRP, P, 4 * 2 * P], BF16, kind="ExternalInput")
    wt_d = nc.dram_tensor("wt", [4, P, 3 * D], BF16, kind="ExternalInput")
    uft_d = nc.dram_tensor("uft", [2, P, D], BF16, kind="ExternalInput")
    biou_d = nc.dram_tensor("biou", [1, 3 * D], BF16, kind="ExternalInput")
    ufb_d = nc.dram_tensor("ufb", [1, D], BF16, kind="ExternalInput")
    srcx_d = nc.dram_tensor("srcx", [P, T], I32, kind="ExternalInput")
    dstl_d = nc.dram_tensor("dstl", [P, T], I32, kind="ExternalInput")
    h_out = nc.dram_tensor("h_out", [NPAD, D], F32, kind="ExternalOutput")
    c_out = nc.dram_tensor("c_out", [NPAD, D], F32, kind="ExternalOutput")
    if DEBUG:
        dbg_d = nc.dram_tensor("dbg", [NB, P, 2 * P], BF16,
                               kind="ExternalOutput")

    # tiles of each block, as (col, size) pairs
    blk_tiles = [[] for _ in range(NB)]
    for col, (ts, b) in enumerate(zip(tile_sizes, tile_block)):
        blk_tiles[b].append((col, ts))

    with tile.TileContext(nc) as tc:
        cst = tc.alloc_tile_pool(name="cst", bufs=1)
        xt_p = tc.alloc_tile_pool(name="xt_p", bufs=2)
        gat_p = tc.alloc_tile_pool(name="gat_p", bufs=6)
        sel_p = tc.alloc_tile_pool(name="sel_p", bufs=4)
        hts_p = tc.alloc_tile_pool(name="hts_p", bufs=4)
        fsb_p = tc.alloc_tile_pool(name="fsb_p", bufs=4)
        fcs_p = tc.alloc_tile_pool(name="fcs_p", bufs=4)
        hsum_p = tc.alloc_tile_pool(name="hsum_p", bufs=3)
        app_p = tc.alloc_tile_pool(name="app_p", bufs=2)
        seg_ps = tc.alloc_tile_pool(name="seg_ps", bufs=1, space="PSUM")
        ftr_ps = tc.alloc_tile_pool(name="ftr_ps", bufs=1, space="PSUM")
        iu_ps = tc.alloc_tile_pool(name="iu_ps", bufs=2, space="PSUM")
        o_ps = tc.alloc_tile_pool(name="o_ps", bufs=2, space="PSUM")

        # ---- constants ----
        wt_sb = cst.tile([P, 4, 3 * D], BF16)
        nc.sync.dma_start(out=wt_sb[:], in_=wt_d[:, :, :].rearrange("k p m -> p k m"))
        uft_sb = cst.tile([P, 2, D], BF16)
        nc.sync.dma_start(out=uft_sb[:], in_=uft_d[:, :, :].rearrange("k p m -> p k m"))
        srcx_sb = cst.tile([P, T], I32)
        nc.sync.dma_start(out=srcx_sb[:], in_=srcx_d[:, :])
        dstl_sb = cst.tile([P, T], I32)
        nc.sync.dma_start(out=dstl_sb[:], in_=dstl_d[:, :])
        ident = cst.tile([P, P], BF16)
        make_identity(nc, ident[:])
        iota = cst.tile([P, P], I32)
        nc.gpsimd.iota(iota[:], pattern=[[1, P]], base=0, channel_multiplier=0)
        if has_biou:
            biou_sb = cst.tile([1, 3 * D], BF16)
            nc.sync.dma_start(out=biou_sb[:], in_=biou_d[:, :])
        if has_ufb:
            ufb_sb = cst.tile([1, D], BF16)
            nc.sync.dma_start(out=ufb_sb[:], in_=ufb_d[:, :])
        if has_biou or has_ufb:
            ones = cst.tile([1, P], BF16)
            nc.gpsimd.memset(ones[:], 1.0)

        # ---- main loop ----
        for g in range(NGRP):
            if g % 2 == 0:
                xt_sb = xt_p.tile([P, 4, 2, P], BF16)
                nc.sync.dma_start(
                    out=xt_sb[:],
                    in_=xt_d[g // 2, :, :].rearrange("p (b k n) -> p b k n", b=4, k=2),
                )
            iu = iu_ps.tile([P, 2, 2 * D], F32, space="PSUM")
            ou = o_ps.tile([P, 2, D], F32, space="PSUM")
            sig_i = app_p.tile([P, 2, D], F32, tag="sig_i")
            tanh_u = app_p.tile([P, 2, D], F32, tag="tanh_u")
            cn = app_p.tile([P, 2, D], F32, tag="cn")
            hn = app_p.tile([P, 2, D], F32, tag="hn")
            cagg = app_p.tile([P, 2, D], F32, tag="cagg")

            for bb in range(2):
                b = g * 2 + bb
                seg = seg_ps.tile([P, 2 * D], F32, space="PSUM", tag="seg")
                ntile = len(blk_tiles[b])
                for ti, (col, tsz) in enumerate(blk_tiles[b]):
                    first = ti == 0
                    last = ti == ntile - 1
                    gath = gat_p.tile([P, 2 * D], BF16, tag="gath")
                    nc.gpsimd.indirect_dma_start(
                        out=gath[0:tsz, :], out_offset=None, in_=hc_d[:, :],
                        in_offset=bass.IndirectOffsetOnAxis(
                            ap=srcx_sb[0:tsz, col:col + 1], axis=0),
                    )
                    sel = sel_p.tile([P, P], BF16, tag="sel")
                    nc.vector.tensor_tensor(
                        out=sel[0:tsz, :],
                        in0=dstl_sb[0:tsz, col:col + 1].to_broadcast([tsz, P]),
                        in1=iota[0:tsz, :],
                        op=mybir.AluOpType.is_equal,
                    )
                    # f/tr bank: cols 0:256 f32 f_pre; 256:384 raw = bf16 tr
                    ftr = ftr_ps.tile([P, 384], F32, space="PSUM", tag="ftr")
                    trv = ftr[:, 256:384].bitcast(BF16)  # [P, 256] bf16
                    for kk in range(2):
                        nc.tensor.transpose(
                            out=trv[:, kk * P:kk * P + tsz],
                            in_=gath[0:tsz, kk * P:(kk + 1) * P],
                            identity=ident[0:tsz, 0:tsz],
                        )
                    hts = hts_p.tile([P, 2, P], BF16, tag="hts")
                    nc.vector.tensor_copy(
                        out=hts[:, :, 0:tsz],
                        in_=trv.rearrange("p (k e) -> p k e", k=2)[:, :, 0:tsz])
                    for kk in range(2):
                        nc.tensor.matmul(
                            out=ftr[0:tsz, 0:D],
                            lhsT=hts[:, kk, 0:tsz],
                            rhs=uft_sb[:, kk, :],
                            start=(kk == 0), stop=(kk == 1 and not has_ufb),
                            skip_group_check=True,
                        )
                    if has_ufb:
                        nc.tensor.matmul(
                            out=ftr[0:tsz, 0:D], lhsT=ones[0:1, 0:tsz],
                            rhs=ufb_sb[0:1, :], start=False, stop=True,
                            skip_group_check=True,
                        )
                    fsb = fsb_p.tile([P, D], BF16, tag="fsb")
                    nc.scalar.activation(fsb[0:tsz, :], ftr[0:tsz, 0:D], AF.Sigmoid)
                    fcs = fcs_p.tile([P, D], BF16, tag="fcs")
                    nc.vector.tensor_mul(fcs[0:tsz, :], fsb[0:tsz, :],
                                         gath[0:tsz, D:2 * D])
                    # segment matmuls: h_sum^T (feat-major) and c_agg (node-major)
                    # start=True only on the very first matmul into the seg
                    # bank: start clears has_written for the WHOLE bank, so a
                    # second start=True would wipe the other regions'
                    # accumulation state. start=False on clear bits = plain
                    # write + set bit (verified on hw).
                    for fs in range(2):
                        nc.tensor.matmul(
                            out=seg[:, fs * P:(fs + 1) * P],
                            lhsT=gath[0:tsz, fs * P:(fs + 1) * P],
                            rhs=sel[0:tsz, :],
                            start=(first and fs == 0), stop=last,
                            skip_group_check=True,
                        )
                    nc.tensor.matmul(
                        out=seg[:, D:2 * D], lhsT=sel[0:tsz, :],
                        rhs=fcs[0:tsz, :],
                        start=False, stop=last, skip_group_check=True,
                    )
                # block epilogue: evacuate seg bank
                hsum = hsum_p.tile([P, 2, P], BF16, tag="hsum")
                nc.vector.tensor_copy(
                    out=hsum[:],
                    in_=seg[:, 0:D].rearrange("p (k n) -> p k n", k=2))
                nc.vector.tensor_copy(out=cagg[:, bb, :], in_=seg[:, D:2 * D])
                if DEBUG:
                    nc.sync.dma_start(
                        out=dbg_d[b, :, :].rearrange("p (k n) -> p k n", k=2),
                        in_=hsum[:])
                # iou matmuls: 4 stationaries (xT kk=0,1; hsumT kk=2,3)
                nbias = 1 if has_biou else 0
                for kk in range(4):
                    lhsT = (xt_sb[:, (b % 4), kk, :] if kk < 2
                            else hsum[:, kk - 2, :])
                    nc.tensor.matmul(
                        out=iu[:, bb, :], lhsT=lhsT, rhs=wt_sb[:, kk, 0:2 * D],
                        start=(kk == 0), stop=(kk == 3 and not nbias),
                        skip_group_check=True,
                    )
                    nc.tensor.matmul(
                        out=ou[:, bb, :], lhsT=lhsT, rhs=wt_sb[:, kk, 2 * D:3 * D],
                        start=(kk == 0), stop=(kk == 3 and not nbias),
                        skip_group_check=True,
                    )
                if has_biou:
                    nc.tensor.matmul(
                        out=iu[:, bb, :], lhsT=ones[0:1, :],
                        rhs=biou_sb[0:1, 0:2 * D], start=False, stop=True,
                        skip_group_check=True)
                    nc.tensor.matmul(
                        out=ou[:, bb, :], lhsT=ones[0:1, :],
                        rhs=biou_sb[0:1, 2 * D:3 * D], start=False, stop=True,
                        skip_group_check=True)

            # group apply: [P, 2, D] == [128, 512] per op
            nc.scalar.activation(sig_i[:], iu[:, :, 0:D], AF.Sigmoid)
            nc.scalar.activation(tanh_u[:], iu[:, :, D:2 * D], AF.Tanh)
            nc.vector.tensor_mul(cn[:], sig_i[:], tanh_u[:])
            nc.vector.tensor_add(cn[:], cn[:], cagg[:])
            nc.scalar.activation(sig_i[:], ou[:], AF.Sigmoid)  # reuse as sig_o
            nc.scalar.activation(tanh_u[:], cn[:], AF.Tanh)  # reuse as tanh_c
            nc.vector.tensor_mul(hn[:], sig_i[:], tanh_u[:])
            r0 = g * 2 * P
            nc.sync.dma_start(
                out=c_out[r0:r0 + 2 * P, :].rearrange("(b n) d -> n b d", b=2),
                in_=cn[:])
            nc.sync.dma_start(
                out=h_out[r0:r0 + 2 * P, :].rearrange("(b n) d -> n b d", b=2),
                in_=hn[:])

        for p in reversed((cst, xt_p, gat_p, sel_p, hts_p, fsb_p, fcs_p,
                           hsum_p, app_p, seg_ps, ftr_ps, iu_ps, o_ps)):
            p.release()

    nc.compile()
    return nc


def _prepare_inputs(x, h, c, W_iouf, U_iou_W, b_iou, U_f_W, U_f_b, st):
    x = np.asarray(x, np.float32)
    h = np.asarray(h, np.float32)
    c = np.asarray(c, np.float32)
    W_iouf = np.asarray(W_iouf, np.float32)
    U_iou_W = np.asarray(U_iou_W, np.float32)
    b_iou = np.asarray(b_iou, np.float32).reshape(1, 3 * D)
    U_f_W = np.asarray(U_f_W, np.float32)
    U_f_b = np.asarray(U_f_b, np.float32).reshape(1, D)

    hc = np.concatenate([h, c], axis=1).astype(BF)

    # gate order reshuffled to [i, u, o]
    perm = np.concatenate([np.arange(0, D), np.arange(2 * D, 3 * D),
                           np.arange(D, 2 * D)])
    Wp = W_iouf[:3 * D][perm]          # [768, 256]
    Up = U_iou_W[perm]                 # [768, 256]
    wt = np.zeros((4, P, 3 * D), np.float32)
    for kk in range(4):
        Wsrc = Wp if kk < 2 else Up
        wt[kk] = Wsrc[:, (kk % 2) * P:(kk % 2) * P + P].T
    wt = wt.astype(BF)
    uft = np.stack([U_f_W[:, 0:P].T, U_f_W[:, P:2 * P].T]).astype(BF)
    biou_p = b_iou[:, perm].astype(BF)
    ufb = U_f_b.astype(BF)

    xpad = np.zeros((NC, NPAD, D), np.float32)
    xpad[:, :NPC] = x.reshape(NC, NPC, D)
    xt = xpad.reshape(NC, XGRP, 4, P, 2, P).transpose(0, 1, 5, 2, 4, 3)
    xt = np.ascontiguousarray(xt).reshape(NC, XGRP, P, 4 * 2 * P).astype(BF)

    in_maps = []
    for k in range(NC):
        in_maps.append({
            "hc": hc,
            "xt": xt[k],
            "wt": wt,
            "uft": uft,
            "biou": biou_p,
            "ufb": ufb,
            "srcx": st["srccols"][k],
            "dstl": st["dstcols"][k],
        })
    return in_maps, (not np.all(b_iou == 0)), (not np.all(U_f_b == 0))


def kernel(x, h, c, src, dst, W_iouf, U_iou_W, b_iou, U_f_W, U_f_b):
    st = _build_edge_structure(src, dst)
    in_maps, has_biou, has_ufb = _prepare_inputs(
        x, h, c, W_iouf, U_iou_W, b_iou, U_f_W, U_f_b, st)
    nc = _build_bass(st["T"], st["tile_sizes"], st["tile_block"],
                     has_biou, has_ufb)
    res = run_bass_kernel_spmd(nc, in_maps, core_ids=list(range(NC)))
    h_new = np.concatenate([res.results[k]["h_out"][:NPC] for k in range(NC)])
    c_new = np.concatenate([res.results[k]["c_out"][:NPC] for k in range(NC)])
    return h_new, c_new
